# revision 2
# baseline (speedup 1.0000x reference)
"""Trainium2 Bass kernel for CosineSimilarityLoss.

Reference math (see problem):
    x1 = a[labels]; x2 = b[labels]          # gather rows, [N, D]
    ip = sum(x1*x2, -1); w1 = ||x1||; w2 = ||x2||
    cos = ip / max(w1*w2, 1e-8)
    mask = cos >= 0.1
    out = sum(cos[mask]) / max(count(mask), 1)

Sharding: rows of a/b are sharded across 8 cores (12500 rows each). The
host partitions `labels` by owning shard; each core gathers only its local
rows via indirect DMA, computes the masked partial sum and count, and the
host combines the 8 (sum, count) pairs.

Per-core device kernel:
  - inputs:  ab  [12500, 1024] f32  (concat of a_shard | b_shard, so one
             gather descriptor fetches both rows)
             idx [128, NT] i32  (local row ids, grid position (p,c) holds
             label c*128+p; pads point at row 0)
             w   [128, NT] f32  (1.0 for real labels, 0.0 for pads)
  - output:  out [1, 2] f32  = (masked partial sum, partial count)
"""

import math
import sys

import numpy as np

if "/opt/trn_rl_repo" not in sys.path:
    sys.path.append("/opt/trn_rl_repo")

V = 100000
D = 512
N_CORES = 8
R = V // N_CORES
P = 128
EPS = 1e-8
MIN_THRESH = 0.1

_CACHE: dict = {}


def _build_program(nt: int, rows: int = R, d: int = D):
    import concourse.bacc as bacc
    import concourse.bass as bass
    import concourse.mybir as mybir
    import concourse.tile as tile

    f32 = mybir.dt.float32
    Alu = mybir.AluOpType
    Act = mybir.ActivationFunctionType

    nc = bacc.Bacc(
        "TRN2", target_bir_lowering=False, debug=False, num_devices=N_CORES
    )
    ab = nc.dram_tensor("ab", [rows, 2 * d], f32, kind="ExternalInput").ap()
    idx = nc.dram_tensor("idx", [P, nt], mybir.dt.int32, kind="ExternalInput").ap()
    wv = nc.dram_tensor("w", [P, nt], f32, kind="ExternalInput").ap()
    out = nc.dram_tensor("out", [1, 2], f32, kind="ExternalOutput").ap()

    with tile.TileContext(nc) as tc:
        with (
            tc.tile_pool(name="persist", bufs=1) as persist,
            tc.tile_pool(name="gather", bufs=4) as gpool,
            tc.tile_pool(name="scrv", bufs=2) as scrv,
            tc.tile_pool(name="scra", bufs=2) as scra,
            tc.tile_pool(name="tail", bufs=1) as tailp,
            tc.tile_pool(name="psum", bufs=1, space="PSUM") as psump,
        ):
            idx_sb = persist.tile([P, nt], mybir.dt.int32)
            w_sb = persist.tile([P, nt], f32)
            ip_sb = persist.tile([P, nt], f32)
            n1_sb = persist.tile([P, nt], f32)
            n2_sb = persist.tile([P, nt], f32)
            nc.sync.dma_start(out=idx_sb[:], in_=idx)
            nc.sync.dma_start(out=w_sb[:], in_=wv)

            for c in range(nt):
                g = gpool.tile([P, 2 * d], f32, tag="g")
                nc.gpsimd.indirect_dma_start(
                    out=g[:],
                    out_offset=None,
                    in_=ab,
                    in_offset=bass.IndirectOffsetOnAxis(
                        ap=idx_sb[:, c : c + 1], axis=0
                    ),
                )
                av = g[:, 0:d]
                bv = g[:, d : 2 * d]
                # DVE: ip = reduce(a*b)  (fused DVE accum ops crash the exec
                # unit on this stack, so multiply + reduce as two ops)
                so = scrv.tile([P, d], f32, tag="scrv")
                nc.vector.tensor_tensor(out=so[:], in0=av, in1=bv, op=Alu.mult)
                nc.vector.tensor_reduce(
                    ip_sb[:, c : c + 1], so[:], axis=mybir.AxisListType.X, op=Alu.add
                )
                # ACT: n1/n2 via Square activation with fused accumulation
                sa = scra.tile([P, d], f32, tag="scra")
                nc.scalar.activation(
                    sa[:], av, Act.Square, accum_out=n1_sb[:, c : c + 1]
                )
                sa2 = scra.tile([P, d], f32, tag="scra")
                nc.scalar.activation(
                    sa2[:], bv, Act.Square, accum_out=n2_sb[:, c : c + 1]
                )

            # tail: cos = ip / max(sqrt(n1)*sqrt(n2), eps); masked sum + count
            w1 = tailp.tile([P, nt], f32)
            nc.scalar.activation(w1[:], n1_sb[:], Act.Sqrt)
            w2 = tailp.tile([P, nt], f32)
            nc.scalar.activation(w2[:], n2_sb[:], Act.Sqrt)
            w12 = tailp.tile([P, nt], f32)
            nc.vector.tensor_tensor(out=w12[:], in0=w1[:], in1=w2[:], op=Alu.mult)
            den = tailp.tile([P, nt], f32)
            nc.vector.tensor_scalar(
                out=den[:], in0=w12[:], scalar1=EPS, scalar2=None, op0=Alu.max
            )
            rec = tailp.tile([P, nt], f32)
            nc.vector.reciprocal(rec[:], den[:])
            cosv = tailp.tile([P, nt], f32)
            nc.vector.tensor_tensor(out=cosv[:], in0=ip_sb[:], in1=rec[:], op=Alu.mult)
            mk = tailp.tile([P, nt], f32)
            nc.vector.tensor_scalar(
                out=mk[:], in0=cosv[:], scalar1=MIN_THRESH, scalar2=None, op0=Alu.is_ge
            )
            mw = tailp.tile([P, nt], f32)
            nc.vector.tensor_tensor(out=mw[:], in0=mk[:], in1=w_sb[:], op=Alu.mult)
            mc = tailp.tile([P, nt], f32)
            nc.vector.tensor_tensor(out=mc[:], in0=cosv[:], in1=mw[:], op=Alu.mult)

            sc = tailp.tile([P, 2], f32)
            nc.vector.tensor_reduce(
                sc[:, 0:1], mc[:], axis=mybir.AxisListType.X, op=Alu.add
            )
            nc.vector.tensor_reduce(
                sc[:, 1:2], mw[:], axis=mybir.AxisListType.X, op=Alu.add
            )
            ones = tailp.tile([P, 1], f32)
            nc.vector.memset(ones[:], 1.0)
            ps = psump.tile([1, 2], f32, space="PSUM")
            nc.tensor.matmul(ps[:], lhsT=ones[:], rhs=sc[:], start=True, stop=True)
            osb = tailp.tile([1, 2], f32)
            nc.vector.tensor_copy(out=osb[:], in_=ps[:])
            nc.sync.dma_start(out=out, in_=osb[:])

    nc.compile()
    return nc


def _get_program(nt: int):
    key = ("prog", nt)
    if key not in _CACHE:
        _CACHE[key] = _build_program(nt)
    return _CACHE[key]


def _shard_host(a, b, labels):
    """Partition labels by owning row-shard; build per-core inputs."""
    a = np.ascontiguousarray(np.asarray(a, dtype=np.float32))
    b = np.ascontiguousarray(np.asarray(b, dtype=np.float32))
    lab = np.asarray(labels).astype(np.int64).ravel()

    locs = []
    for dcore in range(N_CORES):
        lo = dcore * R
        sel = lab[(lab >= lo) & (lab < lo + R)] - lo
        locs.append(np.sort(sel).astype(np.int32))
    kmax = max(len(s) for s in locs)
    nt = max(1, math.ceil(kmax / P))
    kpad = nt * P

    in_maps = []
    for dcore in range(N_CORES):
        lo = dcore * R
        loc = locs[dcore]
        idx_flat = np.zeros(kpad, dtype=np.int32)
        idx_flat[: len(loc)] = loc
        w_flat = np.zeros(kpad, dtype=np.float32)
        w_flat[: len(loc)] = 1.0
        # grid position (p, c) holds flat slot c*128+p
        idx2d = np.ascontiguousarray(idx_flat.reshape(nt, P).T)
        w2d = np.ascontiguousarray(w_flat.reshape(nt, P).T)
        ab = np.concatenate([a[lo : lo + R], b[lo : lo + R]], axis=1)
        in_maps.append(
            {"ab": np.ascontiguousarray(ab), "idx": idx2d, "w": w2d}
        )
    return nt, in_maps


def run_sharded(a, b, labels, **run_kwargs):
    """Shard, run on 8 cores, return (result_scalar, BassKernelResults)."""
    from concourse.bass_utils import run_bass_kernel_spmd

    nt, in_maps = _shard_host(a, b, labels)
    nc = _get_program(nt)
    res = run_bass_kernel_spmd(nc, in_maps, list(range(N_CORES)), **run_kwargs)
    partials = np.stack([r["out"][0] for r in res.results])  # [8, 2]
    total = np.float32(partials[:, 0].astype(np.float64).sum())
    cnt = max(int(round(float(partials[:, 1].sum()))), 1)
    value = np.asarray(np.float32(total) / np.float32(cnt))
    return value, res


def kernel(a, b, labels):
    value, _ = run_sharded(a, b, labels)
    return value
